# revision 1
# baseline (speedup 1.0000x reference)
"""Trainium2 Bass kernel for nn_Brain_17789754540385.

Model: 4 stacked Keras LSTMs (units=3) over (B=8192, T=256) scalar sequences,
then Dense(3->1); output (1, B).

Strategy (pure data parallel, 8 cores, 1024 batch rows each):
- Batch-on-partitions layout: 1024 = 8 groups x 128 partitions per core,
  split into 2 independent streams of 4 groups to hide the cross-engine
  recurrence latency.
- Wavefront over s = 0..258: layer l computes timestep t = s - l. Biases are
  zero, so zero state is a fixed point and wavefront edges need no masking.
- Per stream, per step:
    PE   : transpose the fp16 state slot (128, 64) -> (64, 128) PSUM
    VEC/ACT: copy PSUM -> SBUF (stationary for the matmuls)
    PE   : 4 matmuls, lhsT = [x, H1..H4] rows (13, 128) per group,
           rhs = combined weights (13, 48) -> z (128, 48) fp32 PSUM
    ACT  : tanh over all 48 gate-units (sigmoid folded: sig(x) =
           0.5 + 0.5 tanh(x/2); the 1/2 is folded into the weights)
    VEC  : f = 0.5*tf + 0.5;  Q = f*D;  R = (ti+1)*tg;  D = Q + R  (D = 2c)
    ACT  : tc = tanh(0.5 * D)
    VEC  : H = (to+1)*tc  (H = 2h), written as fp16 into the next state slot
- Stored states: H := 2h, D := 2c. All 0.5 factors folded into the combined
  weight matrix (rows consuming H are halved; f,i,o gate columns halved) and
  the final dense weights.
- Final dense (3 -> 1) and batch gather run on host in fp32.
"""

import numpy as np
import ml_dtypes

BF16 = ml_dtypes.bfloat16

UNITS = 3
N_CORES = 8
B = 8192
T = 256
NG = 8            # batch groups of 128 per core
SLOT = 32         # fp16 comps per group per wavefront slot: [x, H1..H4, pad19]
NSTEP = T + 3     # 259 wavefront steps
NSLOT = NSTEP + 1
NSTREAM = 4
GC = NG // NSTREAM   # groups per stream

_BUILT = {}


# ---------------------------------------------------------------------------
# host-side weight prep
# ---------------------------------------------------------------------------

def _build_wcomb(w, u):
    """Combined stationary-side weight matrix (13, 48) fp16.

    Row 0: x; rows 1+3l .. 3+3l: H_{l+1} (stored H = 2h).
    48 cols = layer-major blocks of 12 = gate-major [f, i, o, g] x 3 units.
    Keras gate order in w/u is i, f, g, o.
    """
    perm = np.r_[UNITS:2 * UNITS, 0:UNITS, 3 * UNITS:4 * UNITS, 2 * UNITS:3 * UNITS]
    wcomb = np.zeros((13, 48), np.float64)
    gate_scale = np.ones(12)
    gate_scale[:9] = 0.5  # sigmoid-as-tanh fold for f, i, o
    for l in range(4):
        wl = np.asarray(w[l], np.float64)[:, perm] * gate_scale
        ul = np.asarray(u[l], np.float64)[:, perm] * gate_scale
        in_scale = 1.0 if l == 0 else 0.5  # layer input is H = 2h for l > 0
        cols = slice(12 * l, 12 * l + 12)
        if l == 0:
            wcomb[0, cols] = (wl * in_scale)[0]
        else:
            wcomb[1 + 3 * (l - 1):1 + 3 * l, cols] = wl * in_scale
        wcomb[1 + 3 * l:4 + 3 * l, cols] = ul * 0.5
    return wcomb.astype(BF16)


# ---------------------------------------------------------------------------
# workarounds: this walrus build allows at most ONE sem wait per instruction
# ---------------------------------------------------------------------------

def _install_patches():
    import concourse.tile as tile_mod
    from concourse import mybir

    if getattr(tile_mod.TileContext, "_wait_split_patched", False):
        return
    from concourse.tile import TileContext, ScopedClock

    orig_commit = TileContext._commit_instruction

    def commit_split(self, inst, lazy_reg_writes: bool = True):
        si = inst.sync_info
        if (
            si is not None
            and len(si.on_wait) > 1
            and inst.engine is not None
            and inst.engine != mybir.EngineType.Unassigned
        ):
            waits = list(si.on_wait)
            for wcond in waits[:-1]:
                nop = mybir.InstNoOp(
                    name=self.nc.get_next_instruction_name(),
                    engine=inst.engine,
                    sync_info=mybir.SyncInfo(on_wait=[wcond], on_update=[]),
                    bass_nofuse=True,
                )
                orig_commit(self, nop, lazy_reg_writes=False)
            si.on_wait = waits[-1:]
            inst.sync_info = si
        return orig_commit(self, inst, lazy_reg_writes)

    def drain_split(self, tick_clock, wait_clock):
        nc = self.nc
        carrier = nc.sync.drain()
        wait_clock.add_sem_waits(
            carrier.ins, ScopedClock({None: tick_clock.global_clock})
        )
        waits = list(carrier.ins.sync_info.on_wait)
        if len(waits) > 1:
            si = carrier.ins.sync_info
            si.on_wait = waits[:1]
            carrier.ins.sync_info = si
            for w in waits[1:]:
                extra = nc.sync.drain()
                extra.ins.sync_info = mybir.SyncInfo(on_wait=[w], on_update=[])
        nc.all_engine_barrier()
        assert self.sems is not None
        popped = nc._tile_sem_poison_stack.pop()
        assert popped is self._sem_poison
        nc.clear_and_free_semaphores(list(self.sems.allocated().values()))
        nc.all_engine_barrier()

    TileContext._commit_instruction = commit_split
    TileContext._drain_and_barrier = drain_split
    TileContext._wait_split_patched = True


# ---------------------------------------------------------------------------
# device kernel build
# ---------------------------------------------------------------------------

def _build_kernel():
    if "nc" in _BUILT:
        return _BUILT["nc"]

    import concourse.bass as bass
    import concourse.tile as tile
    from concourse import mybir

    _install_patches()

    f16 = mybir.dt.bfloat16
    f32 = mybir.dt.float32
    Alu = mybir.AluOpType
    Act = mybir.ActivationFunctionType

    nc = bass.Bass()
    x16_d = nc.declare_dram_parameter("x16", [128, NG * T], f16, isOutput=False)
    wcomb_d = nc.declare_dram_parameter("wcomb", [GC * SLOT, GC * 48], f16, isOutput=False)
    ident_d = nc.declare_dram_parameter("ident", [128, 128], f16, isOutput=False)
    h4_d = nc.declare_dram_parameter("h4", [128, NG * 3], f16, isOutput=True)

    with tile.TileContext(nc) as tc:
        with (
            tc.tile_pool(name="persist", bufs=1) as persist,
            tc.tile_pool(name="work", bufs=2) as work,
            tc.tile_pool(name="st", bufs=2) as stp,
            tc.tile_pool(name="psum_tr", bufs=1, space="PSUM") as ptr,
            tc.tile_pool(name="psum_z", bufs=1, space="PSUM") as pz,
        ):
            SW = GC * SLOT          # slot width per stream (bf16 comps)
            ZW = GC * 48            # z width per stream
            x16 = persist.tile([128, NG * T], f16)
            wcomb = persist.tile([SW, ZW], f16)
            ident = persist.tile([128, 128], f16)
            nc.sync.dma_start(x16[:], x16_d[:])
            nc.sync.dma_start(wcomb[:], wcomb_d[:])
            nc.sync.dma_start(ident[:], ident_d[:])

            S = []
            D = []
            for si_ in range(NSTREAM):
                s_t = persist.tile([128, NSLOT * SW], f16, tag=f"S{si_}", name=f"S{si_}")
                d_t = persist.tile([128, 12 * GC], f32, tag=f"D{si_}", name=f"D{si_}")
                nc.vector.memset(s_t[:], 0.0)
                nc.vector.memset(d_t[:], 0.0)
                S.append(s_t)
                D.append(d_t)

            # x prefill: S[:, slot t, group g, comp 0] = x16[:, goff+g, t]
            x3 = x16.rearrange("p (g t) -> p t g", g=NG)
            npre = min(T, NSLOT - 1)
            for si_ in range(NSTREAM):
                goff = si_ * GC
                s4 = S[si_].rearrange(
                    "p (s g c) -> p s g c", s=NSLOT, g=GC, c=SLOT
                )
                nc.vector.tensor_copy(
                    s4[:, 0:npre, :, 0], x3[:, 0:npre, goff:goff + GC]
                )

            for s in range(NSTEP):
                for si_ in range(NSTREAM):
                    s2 = S[si_].rearrange("p (s f) -> p s f", s=NSLOT)
                    s4 = S[si_].rearrange(
                        "p (s g c) -> p s g c", s=NSLOT, g=GC, c=SLOT
                    )
                    # 1) PE transpose of the current slot
                    tr = ptr.tile([SW, 128], f16, tag=f"tr{si_}", name=f"tr{si_}")
                    nc.tensor.transpose(tr[:], s2[:, s, :], ident[:])
                    # 2) PSUM -> SBUF copy of the stationary (alternate engines)
                    st = stp.tile([SW, 128], f16, tag=f"st{si_}", name=f"st{si_}")
                    if si_ % 2 == 0:
                        nc.vector.tensor_copy(st[:], tr[:])
                    else:
                        nc.scalar.copy(st[:], tr[:])
                    # 3) one block-diagonal matmul for all groups of the stream
                    z = pz.tile([128, ZW], f32, tag=f"z{si_}", name=f"z{si_}")
                    nc.tensor.matmul(
                        z[:], st[:], wcomb[:], start=True, stop=True
                    )
                    # 4) gate tanh
                    G = work.tile([128, ZW], f32, tag=f"G{si_}", name=f"G{si_}")
                    nc.scalar.activation(G[:], z[:], Act.Tanh)
                    g5 = G.rearrange(
                        "p (g l a u) -> p g l a u", g=GC, l=4, a=4, u=3
                    )
                    tf_ = g5[:, :, :, 0, :]
                    ti_ = g5[:, :, :, 1, :]
                    to_ = g5[:, :, :, 2, :]
                    tg_ = g5[:, :, :, 3, :]
                    dv = D[si_].rearrange("p (g l u) -> p g l u", g=GC, l=4, u=3)
                    # 5) VEC chain
                    f_t = work.tile([128, 12 * GC], f32, tag=f"f{si_}", name=f"f{si_}")
                    fv = f_t.rearrange("p (g l u) -> p g l u", g=GC, l=4, u=3)
                    nc.vector.tensor_scalar(fv, tf_, 0.5, 0.5, Alu.mult, Alu.add)
                    q_t = work.tile([128, 12 * GC], f32, tag=f"q{si_}", name=f"q{si_}")
                    qv = q_t.rearrange("p (g l u) -> p g l u", g=GC, l=4, u=3)
                    nc.vector.tensor_mul(qv, fv, dv)
                    r_t = work.tile([128, 12 * GC], f32, tag=f"r{si_}", name=f"r{si_}")
                    rv = r_t.rearrange("p (g l u) -> p g l u", g=GC, l=4, u=3)
                    nc.vector.scalar_tensor_tensor(
                        rv, ti_, 1.0, tg_, Alu.add, Alu.mult
                    )
                    nc.vector.tensor_add(dv, qv, rv)
                    # 6) tc = tanh(0.5 * D)
                    tc_t = work.tile([128, 12 * GC], f32, tag=f"tc{si_}", name=f"tc{si_}")
                    tcv = tc_t.rearrange("p (g l u) -> p g l u", g=GC, l=4, u=3)
                    nc.scalar.activation(tcv, dv, Act.Tanh, scale=0.5)
                    # 7) H = (to+1)*tc into next slot (bf16)
                    hout = s4[:, s + 1, :, 1:13].rearrange(
                        "p g (l u) -> p g l u", l=4, u=3
                    )
                    nc.vector.scalar_tensor_tensor(
                        hout, to_, 1.0, tcv, Alu.add, Alu.mult
                    )

            # output: H4 of the final slot
            h4r = h4_d.rearrange("p (g u) -> p g u", g=NG)
            for si_ in range(NSTREAM):
                goff = si_ * GC
                s4 = S[si_].rearrange(
                    "p (s g c) -> p s g c", s=NSLOT, g=GC, c=SLOT
                )
                nc.sync.dma_start(
                    h4r[:, goff:goff + GC, :], s4[:, NSTEP, :, 10:13]
                )

    _BUILT["nc"] = nc
    return nc


# ---------------------------------------------------------------------------
# entry point
# ---------------------------------------------------------------------------

def kernel(state, w1, u1, b1, w2, u2, b2, w3, u3, b3, w4, u4, b4, wd, bd,
           _want_results=False, _trace=False):
    state = np.asarray(state, np.float32)
    assert state.shape == (B, T), state.shape
    w = [np.asarray(a, np.float32) for a in (w1, w2, w3, w4)]
    u = [np.asarray(a, np.float32) for a in (u1, u2, u3, u4)]
    wd_ = np.asarray(wd, np.float32)
    bd_ = np.asarray(bd, np.float32)

    wc13 = _build_wcomb(w, u)
    wcomb = np.zeros((GC * SLOT, GC * 48), BF16)
    for g in range(GC):
        wcomb[SLOT * g:SLOT * g + 13, 48 * g:48 * g + 48] = wc13
    ident = np.eye(128, dtype=BF16)
    # x16[core, p, g*T + t] = state[1024*core + 128*g + p, t]
    x16 = (
        state.reshape(N_CORES, NG, 128, T)
        .transpose(0, 2, 1, 3)
        .reshape(N_CORES, 128, NG * T)
        .astype(BF16)
    )

    nc = _build_kernel()
    from concourse.bass_utils import run_bass_kernel_spmd

    in_maps = [
        {"x16": x16[c], "wcomb": wcomb, "ident": ident} for c in range(N_CORES)
    ]
    kw = {}
    if _trace:
        kw = dict(trace=True)
    res = run_bass_kernel_spmd(nc, in_maps, list(range(N_CORES)), **kw)

    # gather: h4[c] is (128, NG*3) fp16, H = 2h
    h = np.zeros((B, UNITS), np.float32)
    for c in range(N_CORES):
        hc = np.asarray(res.results[c]["h4"], np.float32).reshape(128, NG, 3)
        # b = 1024c + 128g + p
        h[1024 * c:1024 * (c + 1)] = (
            hc.transpose(1, 0, 2).reshape(1024, 3) * 0.5
        )
    out = (h @ wd_ + bd_)[:, 0][None, :].astype(np.float32)
    if _want_results:
        return out, res
    return out

